# revision 30
# baseline (speedup 1.0000x reference)
"""E3AttentionPooling Trainium2 kernel.

Math (per irrep block b with mul m, deg d):
  q = x W_q / sqrt(m);  k = x W_k / sqrt(m)
  logits[n] = sum_b sum_d q_d^T W_d k_d / sqrt(m^2 d) / sqrt(3) / sqrt(480)
           = sum_b sum_d x_d^T M_b x_d   with  M_b = W_q W_d W_k^T * norm  (folded on host)
  w = exp(logits); out[g] = EquivLinear_Wv(segsum(w x)) / segsum(w)
The Wv transform commutes with the segment sum, so the device only computes
logits, w, and the weighted segment sums; the tiny [G,480] Wv transform and
the division run on host in fp32.

Device layout (per core, atoms sharded contiguously by graph ranges):
  xT  [480, NP] fp8e3  degree-major features on partitions (logits pipeline;
      fp8 is fine here: attention-weight errors attenuate ~1/70 in the output)
  xN  [NP, 482] bf16   natural layout + ones column        (segment pipeline)
  PE: t' = lhsT^T x per 128-feature slice (slices 0-1 use the eigenform
      R^T of the symmetrized bilinear, slices 2-3 the raw M);
  ACT squares / DVE multiplies t' against x -> bf16 prod ; a one-hot-window
  column matmul on PE reduces prod over partitions into per-chunk PSUM
  logits rows ; PE-transpose + ACT exp -> w in atoms-on-partitions layout ;
  DVE builds (iota==batch)*w indicators ; PE segment matmul accumulates
  [128 graphs, 482] in PSUM across all atom tiles (col 480 = norm).
"""
import numpy as np
import ml_dtypes
from contextlib import ExitStack

import concourse.tile as tile
from concourse import bacc, mybir
from concourse.bass_utils import run_bass_kernel_spmd

bf16 = ml_dtypes.bfloat16
fp8 = ml_dtypes.float8_e3m4
F32 = mybir.dt.float32
BF16 = mybir.dt.bfloat16
FP8 = mybir.dt.float8e3

P = 128
C = 512            # atoms per logits chunk (one PSUM row)
RB = 10            # logits rows per macro-block
MBA = C * RB       # atoms per macro-block
TPM = MBA // P     # atom tiles per macro-block
NCORES = 8
GL = 128           # local segment slots (<=127 real + 1 trash)
DF = 480
XNW = 482          # natural row width: 480 feats + ones col + pad
MULS = [128, 64, 32]
DEGS = [1, 3, 5]
SLICE_K = [128, 128, 128, 96]
NSQ = 2            # slices 0..NSQ-1 square on ACT; rest square on DVE

_cache = {}


def _build(NP, loop=None):
    key = (NP, loop)
    if key in _cache:
        return _cache[key]
    NT = NP // P
    NMB = NP // MBA
    nc = bacc.Bacc("TRN2", target_bir_lowering=False, debug=False,
                   num_devices=NCORES)
    xT_d = nc.dram_tensor("xT", [DF, NP], FP8, kind="ExternalInput")
    xN_d = nc.dram_tensor("xN", [NP, XNW], BF16, kind="ExternalInput")
    bat_d = nc.dram_tensor("bat", [P, NT], F32, kind="ExternalInput")
    mblk_d = nc.dram_tensor("mblk", [P, 512], BF16, kind="ExternalInput")
    iota_d = nc.dram_tensor("iota", [P, GL], F32, kind="ExternalInput")
    ident_d = nc.dram_tensor("ident", [P, P], F32, kind="ExternalInput")
    ewin_d = nc.dram_tensor("ewin", [P, 4 * 63], BF16, kind="ExternalInput")
    seg_d = nc.dram_tensor("seg", [P, XNW], F32, kind="ExternalOutput")

    with tile.TileContext(nc) as tc, ExitStack() as ctx:
        const = ctx.enter_context(tc.tile_pool(name="const", bufs=1))
        xtp = ctx.enter_context(tc.tile_pool(name="xtp", bufs=6))
        xnpool = ctx.enter_context(tc.tile_pool(name="xnp", bufs=5))
        sb = ctx.enter_context(tc.tile_pool(name="sb", bufs=6))
        lgp = ctx.enter_context(tc.tile_pool(name="lgp", bufs=2))
        pers = ctx.enter_context(tc.tile_pool(name="pers", bufs=1))
        tpp = ctx.enter_context(tc.tile_pool(name="tpp", bufs=5, space="PSUM"))
        lgps_p = ctx.enter_context(tc.tile_pool(name="lgps", bufs=1, space="PSUM"))
        wtps_p = ctx.enter_context(tc.tile_pool(name="wtps", bufs=1, space="PSUM"))
        segp = ctx.enter_context(tc.tile_pool(name="segp", bufs=1, space="PSUM"))

        mblk_sb = const.tile([P, 512], BF16)
        nc.sync.dma_start(mblk_sb[:], mblk_d.ap())
        iota_sb = const.tile([P, GL], F32)
        nc.sync.dma_start(iota_sb[:], iota_d.ap())
        ident_sb = const.tile([P, P], F32)
        nc.sync.dma_start(ident_sb[:], ident_d.ap())
        ewin_sb = const.tile([P, 4 * 63], BF16)
        nc.sync.dma_start(ewin_sb[:], ewin_d.ap())
        bat_sb = pers.tile([P, NT], F32)
        nc.sync.dma_start(bat_sb[:], bat_d.ap())
        w_sb = pers.tile([P, NT], F32)
        seg_ps = segp.tile([P, XNW], F32)

        import contextlib
        loop_cm = tc.For_i(0, loop, 1) if loop else contextlib.nullcontext()
        with loop_cm:
            body(nc, tc, NMB, NT, xT_d, xN_d, mblk_sb, iota_sb, ident_sb,
                 ewin_sb, bat_sb, w_sb, seg_ps, xtp, xnpool, sb, lgp, tpp,
                 lgps_p, wtps_p)

        seg_sb = pers.tile([P, XNW], F32)
        nc.scalar.copy(seg_sb[:], seg_ps[:])
        nc.sync.dma_start(seg_d.ap(), seg_sb[:])

    nc.compile()
    _cache[key] = nc
    return nc


def body(nc, tc, NMB, NT, xT_d, xN_d, mblk_sb, iota_sb, ident_sb, ewin_sb,
         bat_sb, w_sb, seg_ps, xtp, xnpool, sb, lgp, tpp, lgps_p, wtps_p):
        HB = MBA // 2          # atoms per DMA half-block
        HT = TPM // 2          # atom tiles per half-block
        HC = RB // 2           # chunks per half-block
        for m in range(NMB):
            xts = []           # [half][slice]
            xns = []           # [half]
            for h in range(2):
                a0 = m * MBA + h * HB
                row = []
                for b in range(4):
                    K = SLICE_K[b]
                    t_ = xtp.tile([P, HB], FP8, tag=f"xt{b}")
                    nc.sync.dma_start(
                        t_[:K, :], xT_d.ap()[b * 128:b * 128 + K, a0:a0 + HB])
                    row.append(t_)
                xts.append(row)
                xn_mb = xnpool.tile([P, HT * XNW], BF16, tag="xn")
                src = xN_d.ap()[a0:a0 + HB, :].rearrange("(t p) c -> p t c", p=P)
                dst = xn_mb[:].rearrange("p (t c) -> p t c", t=HT)
                nc.sync.dma_start(dst, src)
                xns.append(xn_mb)

            # steady mblocks: one chain per mblock (RB rows). Last mblock:
            # split per half so its first half's w-chain and segment matmuls
            # overlap the second half's logits work (shortens the tail).
            if m < NMB - 1:
                parts = [(0, 2)]
            else:
                parts = [(0, 1), (1, 2)]
            for h0, h1 in parts:
                rows = (h1 - h0) * HC
                lg_ps = lgps_p.tile([P, C], F32, tag="lg")
                for lcl in range(rows):
                    cl = h0 * HC + lcl
                    for b in range(4):
                        K = SLICE_K[b]
                        h, cll = divmod(cl, HC)
                        xsl = xts[h][b][0:K, cll * C:(cll + 1) * C]
                        tp = tpp.tile([P, C], F32, tag="tp")
                        nc.tensor.matmul(
                            tp[:], mblk_sb[0:K, b * 128:(b + 1) * 128], xsl,
                            start=True, stop=True)
                        prod = sb.tile([P, C], BF16, tag="prod")
                        if b < NSQ:
                            nc.scalar.activation(
                                prod[0:K, :], tp[0:K, :],
                                mybir.ActivationFunctionType.Square)
                        else:
                            nc.vector.tensor_tensor(
                                prod[0:K, :], tp[0:K, :], xsl,
                                mybir.AluOpType.mult)
                        nc.tensor.matmul(
                            lg_ps[0:rows, :],
                            ewin_sb[0:K, b * 63 + 31 - lcl:
                                    b * 63 + 31 - lcl + rows],
                            prod[0:K, :],
                            start=(lcl == 0 and b == 0),
                            stop=(lcl == rows - 1 and b == 3))

                lg_sb = lgp.tile([RB, C], F32, tag="lgsb")
                nc.scalar.copy(lg_sb[0:rows, :], lg_ps[0:rows, :])
                base = m * TPM + h0 * HT
                for k in range(4):
                    wt = wtps_p.tile([P, RB], F32, tag="wt")
                    nc.tensor.transpose(
                        wt[0:P, 0:rows], lg_sb[0:rows, k * 128:(k + 1) * 128],
                        ident_sb[0:rows, 0:rows])
                    nc.scalar.activation(
                        w_sb[:, base + k: base + k + 4 * (rows - 1) + 1: 4],
                        wt[0:P, 0:rows], mybir.ActivationFunctionType.Exp)

                for ltl in range((h1 - h0) * HT):
                    t = base + ltl
                    tl = h0 * HT + ltl
                    h, tll = divmod(tl, HT)
                    aw = sb.tile([P, GL], BF16, tag="aw")
                    nc.vector.tensor_scalar(
                        aw[:], iota_sb[:], bat_sb[:, t:t + 1], w_sb[:, t:t + 1],
                        mybir.AluOpType.is_equal, mybir.AluOpType.mult)
                    nc.tensor.matmul(
                        seg_ps[:], aw[:], xns[h][:, tll * XNW:(tll + 1) * XNW],
                        start=(t == 0), stop=(t == NT - 1))


def _perm():
    idx = []
    off = 0
    for m, d in zip(MULS, DEGS):
        block = np.arange(m * d).reshape(m, d)
        for dd in range(d):
            idx.extend((off + block[:, dd]).tolist())
        off += m * d
    return np.array(idx)


def kernel(**inputs):
    f = np.asarray(inputs["f"], dtype=np.float32)
    batch = np.asarray(inputs["batch"]).astype(np.int64)
    n_graphs = int(np.asarray(inputs["n_graphs"]))
    N, D = f.shape
    assert D == DF

    # fold all normalizations into per-block bilinear matrices
    Ms = []
    for b, (m, d) in enumerate(zip(MULS, DEGS)):
        Wq = np.asarray(inputs[f"Wq{b}"], np.float64)
        Wk = np.asarray(inputs[f"Wk{b}"], np.float64)
        Wd = np.asarray(inputs[f"Wd{b}"], np.float64)
        scale = 1.0 / (m * np.sqrt(m * m * d) * np.sqrt(3.0) * np.sqrt(D))
        Ms.append((Wq @ Wd @ Wk.T) * scale)

    # eigen (square) forms for the first NSQ slices: logits = sum s_r (R x)^2
    Rs, Sg = [], []
    for M_ in Ms:
        sym = (M_ + M_.T) / 2
        lam, U = np.linalg.eigh(sym)
        Rs.append(np.sqrt(np.abs(lam))[:, None] * U.T)
        Sg.append(np.sign(lam))

    # per-slice block-diagonal lhsT [128, 4*128] and sign columns
    # square form: device computes t'_r = sum_i lhsT[i,r] x_i -> lhsT = R^T
    bd_specs = [[Rs[0].T], [Rs[1].T, Rs[1].T], [Ms[1], Ms[2], Ms[2]],
                [Ms[2], Ms[2], Ms[2]]]
    sgns = [Sg[0], np.concatenate([Sg[1], Sg[1]]), np.ones(128), np.ones(96)]
    mblk = np.zeros((P, 512), np.float32)
    for b, spec in enumerate(bd_specs):
        o = 0
        for M_ in spec:
            s_ = M_.shape[0]
            mblk[o:o + s_, b * 128 + o:b * 128 + o + s_] = M_
            o += s_

    perm = _perm()
    fp = f[:, perm]

    # shard by contiguous graph ranges
    counts = np.bincount(batch, minlength=n_graphs)
    cum = np.concatenate([[0], np.cumsum(counts)])
    gsplit = [int(round(c * n_graphs / NCORES)) for c in range(NCORES + 1)]
    asplit = [int(cum[g]) for g in gsplit]
    shard = [asplit[c + 1] - asplit[c] for c in range(NCORES)]
    NP = ((max(max(shard), 1) + MBA - 1) // MBA) * MBA
    NT = NP // P

    iota = np.tile(np.arange(GL, dtype=np.float32)[None, :], (P, 1))
    ident = np.eye(P, dtype=np.float32)
    ewin = np.zeros((P, 4 * 63), np.float32)
    for b in range(4):
        ewin[:len(sgns[b]), b * 63 + 31] = sgns[b]
    consts = {
        "mblk": mblk.astype(bf16), "iota": iota, "ident": ident,
        "ewin": ewin.astype(bf16),
    }

    in_maps = []
    for c in range(NCORES):
        s0, s1 = asplit[c], asplit[c + 1]
        nloc = s1 - s0
        g0 = gsplit[c]
        ng = gsplit[c + 1] - g0
        assert ng <= GL - 1, f"core {c} graph range {ng} > {GL - 1}"
        xT = np.zeros((DF, NP), fp8)
        xT[:, :nloc] = fp[s0:s1].T.astype(fp8)
        xN = np.zeros((NP, XNW), bf16)
        xN[:nloc, :DF] = fp[s0:s1].astype(bf16)
        xN[:nloc, DF] = bf16(1.0)
        bat = np.full(NP, GL - 1, np.float32)
        bat[:nloc] = (batch[s0:s1] - g0).astype(np.float32)
        bat = np.ascontiguousarray(bat.reshape(NT, P).T)
        in_maps.append({"xT": xT, "xN": xN, "bat": bat, **consts})

    nc = _build(NP)
    global _last_in_maps
    _last_in_maps = in_maps
    res = run_bass_kernel_spmd(nc, in_maps, list(range(NCORES)))

    s = np.zeros((n_graphs, DF), np.float64)
    norm = np.zeros(n_graphs, np.float64)
    for c in range(NCORES):
        g0, g1 = gsplit[c], gsplit[c + 1]
        rows = res.results[c]["seg"]
        s[g0:g1] = rows[:g1 - g0, :DF]
        norm[g0:g1] = rows[:g1 - g0, DF]

    # host Wv transform (degree-major -> reference layout) and division
    outb = []
    off = 0
    for b, (m, d) in enumerate(zip(MULS, DEGS)):
        Wv = np.asarray(inputs[f"Wv{b}"], np.float64)
        sb_ = np.stack([s[:, off + dd * m:off + (dd + 1) * m]
                        for dd in range(d)], axis=2)  # [G, m, d]
        vb = np.einsum('gmd,mo->god', sb_, Wv) / np.sqrt(m)
        outb.append(vb.reshape(n_graphs, m * d))
        off += m * d
    num = np.concatenate(outb, axis=1)
    out = num / np.clip(norm, 1e-8, None)[:, None]
    return out.astype(np.float32)
